# revision 77
# baseline (speedup 1.0000x reference)
"""Multi-head attention layer (B=2,S=2048,D=1024,H=16) on 8 TRN2 NeuronCores.

Sharding: data parallel over batch (2) x tensor parallel over heads (4 heads
per core).  Each core computes, for its (batch b, head-group hg):
  QT = (X_b @ Wq[:,cols] + bq + emotion)^T        [256, S]    (n on partitions)
  KT = (Xv_b @ Wk[:,cols])^T                      [256, Skv]  (compacted keys)
  V  = Xv_b @ Wv[:,cols]                          [Skv, 256]  (+ones col)
  scoresT[j,i] = KT_h-slices x QT_h, exp fused with 1/8 scale + key mask bias,
  OT_h = V_h_aug^T @ PT  (row 64 = softmax denominator l via the ones column),
  out_partial = (OT/l)^T @ Wo[rows,:]             [S, D] fp16
Host compacts key/value positions by the attention mask and sums the 4
partial outputs per batch.

Math folds (exact): bk cancels in softmax (per-query constant); bv commutes
through the attention average, so bo' = bo + bv @ Wo is added on host; the
emotion bias is folded into bq host-side.

Matmul operands are bf16 (fp32r trips the HAM duty-cycle throttle); PSUM
accumulation is fp32.  Inputs stream via one batched 3D-AP DMA per tensor
across three DGE rings.  Phase 2 is paced by the scalar-engine exp; the PE
is kept streaming by interleaving V projection (first block), the NEXT query
block's Q projection and the PREVIOUS block's output projection as fine
pending units, one per j iteration.
"""
import math
import sys

sys.path.insert(0, "/opt/trn_rl_repo")

import ml_dtypes
import numpy as np

import concourse.bass as bass
import concourse.tile as tile
from concourse import bacc, mybir
from concourse.bass_utils import run_bass_kernel_spmd

B, S, D, H = 2, 2048, 1024, 16
DH = D // H          # 64
HPC = 4              # heads per core
NCOL = HPC * DH      # 256 columns of Wq/Wk/Wv per core
NC2 = NCOL // 128    # 2 partition-chunks of the head dim
ND = D // 128        # 8 contraction chunks
NI = S // 512        # 4 query 512-chunks
F32 = mybir.dt.float32
F16 = mybir.dt.float16
BF16 = mybir.dt.bfloat16
FP8 = mybir.dt.float8e4
BF16_NP = ml_dtypes.bfloat16
AF = mybir.ActivationFunctionType
AP = bass.AP
PM_DR = mybir.MatmulPerfMode.DoubleRow

_PROGRAM_CACHE = {}


def _chunks(total, step):
    out = []
    o = 0
    while o < total:
        out.append((o, min(step, total - o)))
        o += step
    return out


def build_program(skv: int, debug: bool = False):
    """One NeuronCore's program; SPMD across 8 cores with different data."""
    nj = skv // 128
    nblk = skv // 256          # 256-key DoubleRow blocks
    ntail = (skv - nblk * 256) // 128  # trailing 128-key chunk (0 or 1)
    kchunks = _chunks(skv, 512)
    nc = bacc.Bacc("TRN2", target_bir_lowering=False, debug=debug, num_devices=8)

    xt = nc.declare_dram_parameter("xt", [D, S], BF16, isOutput=False)
    wq = nc.declare_dram_parameter("wq", [D, NCOL], BF16, isOutput=False)
    wk = nc.declare_dram_parameter("wk", [D, NCOL], BF16, isOutput=False)
    wv = nc.declare_dram_parameter("wv", [D, NCOL], BF16, isOutput=False)
    wo = nc.declare_dram_parameter("wo", [NCOL, D], BF16, isOutput=False)
    bqe = nc.declare_dram_parameter("bqe", [NCOL], F32, isOutput=False)
    maskb = nc.declare_dram_parameter("maskb", [NC2, skv], F32, isOutput=False)
    out = nc.declare_dram_parameter("out", [S, D], F16, isOutput=True)

    def ap3(param, ap_list, offset=0):
        p = param.ap()
        return AP(tensor=p.tensor, offset=p.offset + offset, ap=ap_list)

    with tile.TileContext(nc) as tc:
        with tc.tile_pool(name="singles", bufs=1) as singles:
            # --- persistent SBUF tiles -----------------------------------
            twqa = singles.tile([128, ND, NCOL], BF16, tag="wqa", name="twqa")
            twka = singles.tile([128, ND, NCOL], BF16, tag="wka", name="twka")
            twva = singles.tile([128, ND, NCOL], BF16, tag="wva", name="twva")
            twoa = singles.tile([128, NC2, D], BF16, tag="woa", name="twoa")
            # queries are host-permuted so the masked-in keys are columns
            # 0:skv of xt -- K/V projections read a prefix of the same tile
            txta = singles.tile([128, ND, S], BF16, tag="xta", name="txta")
            twq = [twqa[:, d, :] for d in range(ND)]
            twk = [twka[:, d, :] for d in range(ND)]
            twv = [twva[:, d, :] for d in range(ND)]
            two = [twoa[:, c, :] for c in range(NC2)]
            txkv = [txta[:, d, 0:skv] for d in range(ND)]
            txt = [
                [txta[:, d, i * 512:(i + 1) * 512] for i in range(NI)]
                for d in range(ND)
            ]
            tqt = [singles.tile([128, S], BF16, tag=f"qt{c}", name=f"qt{c}") for c in range(NC2)]
            tkt = [singles.tile([128, skv], BF16, tag=f"kt{c}", name=f"kt{c}") for c in range(NC2)]
            # V tiles: [j chunk, head, 80] with col 64 = ones (denominator)
            tva = singles.tile([128, nj, HPC, 80], BF16, tag="tva", name="tva")
            tot = [singles.tile([128, S], F32, tag=f"ot{c}", name=f"ot{c}") for c in range(NC2)]
            totn = [singles.tile([128, S], BF16, tag=f"otn{c}", name=f"otn{c}") for c in range(NC2)]
            # softmax denominators: rows 0/32/64/96 hold heads 0..3; the
            # custom-DVE reciprocal only works on full-width offset-0 APs,
            # so stage the rows and batch it
            tstage = singles.tile([97, S], F32, tag="lstage", name="tstage")
            trecf = singles.tile([97, S], F32, tag="lrecf", name="trecf")
            trecb = singles.tile([97, S], BF16, tag="lrecb", name="trecb")
            tones4 = singles.tile([97, 64], BF16, tag="ones4", name="tones4")
            tonesf = singles.tile([128, 64], F32, tag="onesf", name="tonesf")
            tmbs = singles.tile([128, NC2, nj], F32, tag="mbs", name="tmbs")
            tbq = singles.tile([128, NC2], F32, tag="tbq", name="tbq")

            # --- input DMAs ----------------------------------------------
            # Two ~200GB/s hardware DGE rings (sync, scalar).  Critical
            # chain is K proj (wk + all xtkv d-chunks) then Q proj of block
            # 0 (wq + xt cols 0:512); xtkv streams per-d on both rings so
            # the d-outer K-proj consumes chunks as they land.
            def dma_w(eng, tile_, param):
                eng.dma_start(
                    out=tile_,
                    in_=ap3(param, [[NCOL, 128], [128 * NCOL, ND], [1, NCOL]]),
                )

            def dma_xt(eng, dlo, dhi, clo, chi):
                if chi <= clo:
                    return
                eng.dma_start(
                    out=txta[:, dlo:dhi, clo:chi],
                    in_=ap3(
                        xt,
                        [[S, 128], [128 * S, dhi - dlo], [1, chi - clo]],
                        offset=dlo * 128 * S + clo,
                    ),
                )

            # column-chunk schedule: the first exp needs only wk + xt cols
            # 0:512 + half of wq + wv (~2.25MB over two ~110GB/s rings);
            # later key chunks and query blocks stream in behind it.
            colchunks = [(jo, jw) for jo, jw in kchunks]
            if skv < S:
                colchunks.append((skv, S - skv))
            # wk split by d-half so the first K matmul starts sooner
            nc.sync.dma_start(
                out=twka[:, 4:8, :],
                in_=ap3(wk, [[NCOL, 128], [128 * NCOL, 4], [1, NCOL]],
                        offset=4 * 128 * NCOL),
            )
            nc.sync.dma_start(
                out=twka[:, 0:4, :],
                in_=ap3(wk, [[NCOL, 128], [128 * NCOL, 4], [1, NCOL]]),
            )
            first = True
            for jo, jw in colchunks:
                dma_xt(nc.sync, 0, 4, jo, jo + jw)
                dma_xt(nc.scalar, 4, 8, jo, jo + jw)
                if first:
                    for c in range(NC2):
                        nc.sync.dma_start(
                            out=twqa[:, :, c * 128:(c + 1) * 128],
                            in_=ap3(
                                wq,
                                [[NCOL, 128], [128 * NCOL, ND], [1, 128]],
                                offset=c * 128,
                            ),
                        )
                        if c == 0:
                            dma_w(nc.scalar, twva, wv)
                    first = False
            nc.scalar.dma_start(
                out=twoa, in_=ap3(wo, [[D, 128], [128 * D, NC2], [1, D]])
            )
            # small tiles on the gpsimd (software DGE) queue
            nc.gpsimd.dma_start(
                out=tmbs, in_=ap3(maskb, [[1, 128], [skv, NC2], [128, nj]])
            )
            nc.gpsimd.dma_start(out=tbq, in_=ap3(bqe, [[1, 128], [128, NC2]]))
            tbiasq = [tbq[:, c:c + 1] for c in range(NC2)]

            # HAM pre-warm: the PE clock-gate defaults to 1.2GHz and takes
            # ~3.4us of sustained activity to release; burn dummy matmuls
            # on an uninitialized scratch tile while the input DMAs stream
            # so the K projection starts at 2.4GHz.  Results never read.
            tjunk = singles.tile([128, 512], BF16, tag="junk", name="tjunk")
            nc.vector.memset(tjunk, 1.0)
            nc.vector.memset(tonesf, 1.0)
            nc.vector.memset(tva, 1.0)  # cols 64:80 stay 1.0 -> ones column
            # rows of tstage between the 32h anchors are swept by the batched
            # reciprocal; init once so they stay finite
            nc.vector.memset(tstage, 1.0)
            for h in range(HPC):
                nc.vector.tensor_copy(
                    out=tones4[32 * h:32 * h + 1, :], in_=tonesf[0:1, :]
                )

            # --- phase 2: attention + normalize + output projection ------
            # The j-loop is paced by the ACT-engine exp; one pending unit of
            # independent PE work is popped per j iteration so the PE queue
            # never drains: V projection (early), Q projection of block i+1
            # and the output projection of block i-1.
            with (
                tc.tile_pool(name="pts", bufs=3) as pts,
                tc.tile_pool(name="obuf", bufs=4) as obuf,
                tc.tile_pool(name="ps2", bufs=2, space="PSUM") as ps2,
                tc.tile_pool(name="pot", bufs=2, space="PSUM") as pot,
                tc.tile_pool(name="plf", bufs=2, space="PSUM") as plf,
            ):
                pending = []
                state = {"tail": False}
                obs = {}
                # junk-fill target: a ps2 slot, provably idle until the
                # first scores matmul (never read, so its slot recycles)
                jh = ps2.tile([128, 1024], F32, tag="ps", name="jht")

                def junkfill(n):
                    for _ in range(n):
                        nc.tensor.matmul(
                            jh[0:64, 0:128], tjunk[:, 0:64], tjunk[:, 0:128],
                            start=True, stop=True,
                        )
                dorder = [4, 5, 6, 7, 0, 1, 2, 3]  # DMA arrival order

                def emit_kc(ci, half):
                    # K projection of key chunk ci, d-half (0: d4-7 start,
                    # 1: d0-3 stop + bf16 cast).  Borrows plf slots.
                    jo, jw = kchunks[ci]
                    if half == 0:
                        state[f"kc{ci}"] = [
                            plf.tile([128, 512], F32, tag="plf", name="pkt")
                            for _ in range(NC2)
                        ]
                    kp = state[f"kc{ci}"]
                    for d in dorder[half * 4:half * 4 + 4]:
                        for c in range(NC2):
                            nc.tensor.matmul(
                                kp[c][:, 0:jw],
                                twk[d][:, c * 128:(c + 1) * 128],
                                txkv[d][:, jo:jo + jw],
                                start=(half == 0 and d == dorder[0]),
                                stop=(half == 1 and d == dorder[7]),
                            )
                        if ci == 0:
                            # chunk 0 is DMA-paced: inter-d stalls exceed
                            # the HAM idle window and re-throttle the PE to
                            # 1.2GHz; bridge them with data-free junk MMs
                            junkfill(2)
                    if half == 1:
                        for c in range(NC2):
                            nc.vector.tensor_copy(
                                out=tkt[c][:, jo:jo + jw], in_=kp[c][:, 0:jw]
                            )

                def emit_v(j):
                    # one V-proj unit: 8 accumulating matmuls -> fp8 cast
                    # into the per-head slices of tvd (borrows a plf slot)
                    ps = plf.tile([128, 512], F32, tag="plf", name="pvt")
                    for d in range(ND):
                        nc.tensor.matmul(
                            ps[:, 0:NCOL],
                            txkv[d][:, j * 128:(j + 1) * 128],
                            twv[d],
                            start=(d == 0),
                            stop=(d == ND - 1),
                        )
                    pin = AP(
                        tensor=ps.tensor, offset=ps.offset,
                        ap=[list(ps.ap[0]), [64, HPC], [1, 64]],
                    )
                    nc.vector.tensor_copy(out=tva[:, j, :, 0:64], in_=pin)

                def emit_pf(i, so, n):
                    # one outproj unit: 2 accumulating matmuls -> fp16 copy;
                    # DMA fires once both D-halves of the row block are done.
                    # In the drain phase ps2 is score-free, so odd units
                    # borrow its slots for a 4-deep PSUM pipeline.
                    sidx = i * 4 + so
                    ssl = slice(sidx * 128, (sidx + 1) * 128)
                    nsl = slice(n * 512, (n + 1) * 512)
                    if state["tail"] and so % 2:
                        pf = ps2.tile([128, 1024], F32, tag="ps", name="pft")[:, 0:512]
                    else:
                        pf = plf.tile([128, 512], F32, tag="plf", name="pft")
                    for c in range(NC2):
                        nc.tensor.matmul(
                            pf,
                            totn[c][:, ssl],
                            two[c][:, nsl],
                            start=(c == 0),
                            stop=(c == NC2 - 1),
                        )
                    if n == 0:
                        obs[so] = obuf.tile([128, 1024], F16, tag="ob", name="obt")
                    ob = obs[so]
                    if state["tail"] and (so + n) % 2:
                        # ACT is exp-free in the tail; split the copies so
                        # outproj units pipeline two-wide
                        nc.scalar.copy(out=ob[:, nsl], in_=pf)
                    else:
                        nc.vector.tensor_copy(out=ob[:, nsl], in_=pf)
                    if n == 1:
                        # odd blocks go to the gpsimd ring (scalar queue is
                        # busy with exp) except in the drain, where the
                        # scalar queue is free and gpsimd drains slowly
                        if sidx % 2 == 0:
                            ring = nc.sync
                        else:
                            ring = nc.scalar if state["tail"] else nc.gpsimd
                        ring.dma_start(out=out[ssl, :], in_=ob)

                def emit_norm(i):
                    # batched reciprocal + bf16 cast of 1/l, then per-head
                    # ones-row broadcast + normalize (deferred off the block
                    # boundary into later j-loop pop slots)
                    isl = slice(i * 512, (i + 1) * 512)
                    nc.vector.reciprocal_approx_fast(
                        out=trecf[:, isl], in_=tstage[:, isl]
                    )
                    nc.vector.tensor_copy(out=trecb[:, isl], in_=trecf[:, isl])
                    for c in range(NC2):
                        hA, hB = 2 * c, 2 * c + 1
                        plA = plf.tile([64, 512], F32, tag="plf", name="plA")
                        plB = plf.tile([64, 512], F32, tag="plf", name="plB")
                        nc.tensor.matmul(
                            plA,
                            tones4[32 * hA:32 * hA + 1, :],
                            trecb[32 * hA:32 * hA + 1, isl],
                            start=True,
                            stop=True,
                            tile_position=(32 * hA, 0),
                        )
                        nc.tensor.matmul(
                            plB,
                            tones4[32 * hB:32 * hB + 1, :],
                            trecb[32 * hB:32 * hB + 1, isl],
                            start=True,
                            stop=True,
                            tile_position=(32 * hB, 0),
                        )
                        nc.vector.tensor_mul(
                            out=totn[c][0:64, isl], in0=tot[c][0:64, isl], in1=plA
                        )
                        nc.vector.tensor_mul(
                            out=totn[c][64:128, isl], in0=tot[c][64:128, isl], in1=plB
                        )

                def emit_q(i, c):
                    # one Q-proj unit: 8 accumulating matmuls (borrowing a
                    # ps2 tile; scores j+1 uses the other buffer) + bias add
                    isl = slice(i * 512, (i + 1) * 512)
                    ps = ps2.tile([128, 1024], F32, tag="ps", name="pqt")
                    for d in range(ND):
                        nc.tensor.matmul(
                            ps[:, 0:512],
                            twq[d][:, c * 128:(c + 1) * 128],
                            txt[d][i],
                            start=(d == 0),
                            stop=(d == ND - 1),
                        )
                    nc.vector.tensor_scalar_add(
                        out=tqt[c][:, isl], in0=ps[:, 0:512], scalar1=tbiasq[c]
                    )

                # K projection of key chunk 0, V projection of chunks 0-3
                # and Q projection of (c=0, block 0) run before the first
                # j-loop; later K chunks, V chunks and the c=1 Q projection
                # interleave into (c=0, i=0)'s pop slots as they stream in.
                # Slot deadlines: kc(ci) before the j-loop reaches its first
                # key chunk; V(j) before iteration j (attnV j fires at j+1).
                pw = plf.tile([128, 512], F32, tag="plf", name="pwt")
                for _ in range(70):
                    nc.tensor.matmul(
                        pw[0:64, 0:64], tjunk[:, 0:64], tjunk[:, 0:64],
                        start=True, stop=True,
                    )
                emit_kc(0, 0)
                emit_kc(0, 1)
                junkfill(4)  # bridge the wv wait
                for j in range(min(5, nj)):
                    emit_v(j)
                junkfill(4)  # bridge the wq/xt-block-0 wait
                emit_q(0, 0)
                # warm slots with deadlines (slot k pops at end of c=0
                # iteration k, after attnV k-1): kc(ci) before its first
                # scores use; V(j) at slot <= j-1; Q0c1 last slot.
                warm = []
                for ci in range(1, len(kchunks)):
                    warm.append(lambda ci=ci: emit_kc(ci, 0))
                    warm.append(lambda ci=ci: emit_kc(ci, 1))
                wv_rest = [lambda j=j: emit_v(j) for j in range(5, nj)]
                if len(kchunks) == 3:
                    # interleave: [kc1a, kc1b, V5, V6, kc2a, kc2b, V7, ...]
                    head, tail2 = warm[:2], warm[2:]
                    warm = head + wv_rest[:2] + tail2 + wv_rest[2:]
                elif len(kchunks) > 3:
                    # degenerate skv (all-ones mask): run everything inline
                    for u in warm + wv_rest:
                        u()
                    warm = []
                else:
                    warm += wv_rest
                warm.append(lambda: emit_q(0, 1))

                for i in range(NI):
                    isl = slice(i * 512, (i + 1) * 512)
                    if i + 1 < NI:
                        # front of the queue: Q projections are the only
                        # dependency-free PE work at the block boundary,
                        # where scores stall ~800ns on the exp/ps2 chain
                        for c in reversed(range(NC2)):
                            pending.insert(
                                0, (i + 1, lambda i=i, c=c: emit_q(i + 1, c))
                            )
                    # force-emit units whose results this block depends on
                    # (Q projection of block i) if the queue hasn't drained
                    while any(dl <= i for dl, _ in pending):
                        pending.pop(0)[1]()
                    for c in range(NC2):
                        if i > 0 or c > 0:
                            while warm:
                                warm.pop(0)()
                        hA, hB = 2 * c, 2 * c + 1
                        potA = pot.tile([DH + 1, 512], F32, tag="pot", name="pott")
                        potB = pot.tile([DH + 1, 512], F32, tag="pot", name="pott")
                        pts_hist = []
                        for j in range(nj):
                            pscore = ps2.tile([128, 1024], F32, tag="ps", name="pscore")
                            nc.tensor.matmul(
                                pscore[:, 0:512],
                                tkt[c][0:64, j * 128:(j + 1) * 128],
                                tqt[c][0:64, isl],
                                start=True,
                                stop=True,
                                tile_position=(0, 0),
                            )
                            nc.tensor.matmul(
                                pscore[:, 512:1024],
                                tkt[c][64:128, j * 128:(j + 1) * 128],
                                tqt[c][64:128, isl],
                                start=True,
                                stop=True,
                                tile_position=(64, 0),
                            )
                            pt = pts.tile([128, 1024], BF16, tag="pt", name="ptile")
                            nc.scalar.activation(
                                out=pt,
                                in_=pscore,
                                func=AF.Exp,
                                bias=tmbs[:, c, j:j + 1],
                                scale=1.0 / math.sqrt(DH),
                            )
                            pts_hist.append(pt)
                            if j > 0:
                                pprev = pts_hist[j - 1]
                                nc.tensor.matmul(
                                    potA, tva[:, j - 1, hA, 0:65], pprev[:, 0:512],
                                    start=(j - 1 == 0), stop=False,
                                )
                                nc.tensor.matmul(
                                    potB, tva[:, j - 1, hB, 0:65], pprev[:, 512:1024],
                                    start=(j - 1 == 0), stop=False,
                                )
                            if warm:
                                warm.pop(0)()
                            elif pending:
                                pending.pop(0)[1]()
                        # one pop slot before the final pair: they wait on
                        # the last exp, so hand the PE independent work
                        if pending:
                            pending.pop(0)[1]()
                        nc.tensor.matmul(
                            potA, tva[:, nj - 1, hA, 0:65], pts_hist[nj - 1][:, 0:512],
                            start=(nj == 1), stop=True,
                        )
                        nc.tensor.matmul(
                            potB, tva[:, nj - 1, hB, 0:65], pts_hist[nj - 1][:, 512:1024],
                            start=(nj == 1), stop=True,
                        )
                        nc.vector.tensor_copy(out=tot[c][0:64, isl], in_=potA[0:DH, :])
                        nc.vector.tensor_copy(out=tot[c][64:128, isl], in_=potB[0:DH, :])
                        nc.vector.tensor_copy(
                            out=tstage[32 * hA:32 * hA + 1, isl],
                            in_=potA[DH:DH + 1, :],
                        )
                        nc.vector.tensor_copy(
                            out=tstage[32 * hB:32 * hB + 1, isl],
                            in_=potB[DH:DH + 1, :],
                        )
                    pending.append((NI + 1, lambda i=i: emit_norm(i)))
                    for so in range(4):
                        for n in range(2):
                            pending.append(
                                (NI + 1, lambda i=i, so=so, n=n: emit_pf(i, so, n))
                            )
                state["tail"] = True
                # the final norm chain leaves the PE idle ~4us, which
                # re-throttles HAM (K=4/8) and halves the drain's matmul
                # clock; after the norm unit is queued, keep the clock-gate
                # open with a few junk matmuls through the free pot slots
                for _ in range(6):
                    jp = plf.tile([128, 512], F32, tag="plf", name="jpt")
                    nc.tensor.matmul(
                        jp[0:64, :], tjunk[:, 0:64], tjunk, start=True, stop=True
                    )
                while pending:
                    pending.pop(0)[1]()

    nc.compile()
    return nc


def _get_program(skv):
    if skv not in _PROGRAM_CACHE:
        _PROGRAM_CACHE[skv] = build_program(skv)
    return _PROGRAM_CACHE[skv]


def _shard_inputs(hidden_states, attention_mask, Wq, bq, Wk, bk, Wv, bv,
                  emotion_w, Wo, bo):
    hs = np.asarray(hidden_states, dtype=np.float32)
    mask = np.asarray(attention_mask)
    Wq = np.asarray(Wq, dtype=np.float32)
    Wk = np.asarray(Wk, dtype=np.float32)
    Wv = np.asarray(Wv, dtype=np.float32)
    Wo = np.asarray(Wo, dtype=np.float32)
    bq = np.asarray(bq, dtype=np.float32)
    bv = np.asarray(bv, dtype=np.float32)
    bo = np.asarray(bo, dtype=np.float32)
    ew = np.asarray(emotion_w, dtype=np.float32)

    idx = [np.nonzero(mask[b])[0] for b in range(B)]
    sv = max(len(ix) for ix in idx)
    skv = max(128, ((sv + 127) // 128) * 128)



    in_maps = []
    perms = []
    for b in range(B):
        # permute queries so masked-in keys occupy columns 0:sv -- the K/V
        # projections then read a prefix of xt and no separate compacted
        # tensor is shipped.  Host inverse-permutes the output rows.
        rest = np.nonzero(mask[b] == 0)[0]
        perm = np.concatenate([idx[b], rest])
        perms.append(perm)
        xt_b = np.ascontiguousarray(hs[b][perm].T.astype(BF16_NP))  # [D, S]
        # -4 shift (softmax-invariant) keeps bf16 exp outputs small
        maskb_b = np.full((NC2, skv), -4.0, dtype=np.float32)
        maskb_b[:, len(idx[b]):] = -1e30
        for hg in range(H // HPC):
            cols = slice(hg * NCOL, (hg + 1) * NCOL)
            bqe = bq[cols] + ew[hg * HPC:(hg + 1) * HPC].reshape(NCOL)
            in_maps.append(
                {
                    "xt": xt_b,
                    "wq": np.ascontiguousarray(Wq[:, cols].astype(BF16_NP)),
                    "wk": np.ascontiguousarray(Wk[:, cols].astype(BF16_NP)),
                    "wv": np.ascontiguousarray(Wv[:, cols].astype(BF16_NP)),
                    "wo": np.ascontiguousarray(Wo[cols, :].astype(BF16_NP)),
                    "bqe": np.ascontiguousarray(bqe),
                    "maskb": maskb_b,
                }
            )
    # bk cancels in softmax; bv rides through the attention average:
    # out = attn @ (XWv) @ Wo + (bv @ Wo + bo)
    bo_adj = (bo.astype(np.float64) + bv.astype(np.float64) @ Wo.astype(np.float64))
    return in_maps, skv, bo_adj, perms


def run(inputs, trace=False, trace_kwargs=None):
    in_maps, skv, bo_adj, perms = _shard_inputs(**inputs)
    nc = _get_program(skv)
    res = run_bass_kernel_spmd(
        nc,
        in_maps,
        core_ids=list(range(8)),
        trace=trace,
        **(trace_kwargs or {}),
    )
    out = np.zeros((B, S, D), dtype=np.float32)
    for b in range(B):
        acc = np.zeros((S, D), dtype=np.float64)
        for hg in range(4):
            acc += res.results[b * 4 + hg]["out"].astype(np.float64)
        out[b][perms[b]] = (acc + bo_adj).astype(np.float32)
    return out, res


def kernel(**inputs):
    out, _ = run(inputs, trace=False)
    return out


# revision 78
# speedup vs baseline: 1.0152x; 1.0152x over previous
"""Multi-head attention layer (B=2,S=2048,D=1024,H=16) on 8 TRN2 NeuronCores.

Sharding: data parallel over batch (2) x tensor parallel over heads (4 heads
per core).  Each core computes, for its (batch b, head-group hg):
  QT = (X_b @ Wq[:,cols] + bq + emotion)^T        [256, S]    (n on partitions)
  KT = (Xv_b @ Wk[:,cols])^T                      [256, Skv]  (compacted keys)
  V  = Xv_b @ Wv[:,cols]                          [Skv, 256]  (+ones col)
  scoresT[j,i] = KT_h-slices x QT_h, exp fused with 1/8 scale + key mask bias,
  OT_h = V_h_aug^T @ PT  (row 64 = softmax denominator l via the ones column),
  out_partial = (OT/l)^T @ Wo[rows,:]             [S, D] fp16
Host compacts key/value positions by the attention mask and sums the 4
partial outputs per batch.

Math folds (exact): bk cancels in softmax (per-query constant); bv commutes
through the attention average, so bo' = bo + bv @ Wo is added on host; the
emotion bias is folded into bq host-side.

Matmul operands are bf16 (fp32r trips the HAM duty-cycle throttle); PSUM
accumulation is fp32.  Inputs stream via one batched 3D-AP DMA per tensor
across three DGE rings.  Phase 2 is paced by the scalar-engine exp; the PE
is kept streaming by interleaving V projection (first block), the NEXT query
block's Q projection and the PREVIOUS block's output projection as fine
pending units, one per j iteration.
"""
import math
import sys

sys.path.insert(0, "/opt/trn_rl_repo")

import ml_dtypes
import numpy as np

import concourse.bass as bass
import concourse.tile as tile
from concourse import bacc, mybir
from concourse.bass_utils import run_bass_kernel_spmd

B, S, D, H = 2, 2048, 1024, 16
DH = D // H          # 64
HPC = 4              # heads per core
NCOL = HPC * DH      # 256 columns of Wq/Wk/Wv per core
NC2 = NCOL // 128    # 2 partition-chunks of the head dim
ND = D // 128        # 8 contraction chunks
NI = S // 512        # 4 query 512-chunks
F32 = mybir.dt.float32
F16 = mybir.dt.float16
BF16 = mybir.dt.bfloat16
FP8 = mybir.dt.float8e4
BF16_NP = ml_dtypes.bfloat16
AF = mybir.ActivationFunctionType
AP = bass.AP
PM_DR = mybir.MatmulPerfMode.DoubleRow

_PROGRAM_CACHE = {}


def _chunks(total, step):
    out = []
    o = 0
    while o < total:
        out.append((o, min(step, total - o)))
        o += step
    return out


def build_program(skv: int, debug: bool = False):
    """One NeuronCore's program; SPMD across 8 cores with different data."""
    nj = skv // 128
    nblk = skv // 256          # 256-key DoubleRow blocks
    ntail = (skv - nblk * 256) // 128  # trailing 128-key chunk (0 or 1)
    kchunks = _chunks(skv, 512)
    nc = bacc.Bacc("TRN2", target_bir_lowering=False, debug=debug, num_devices=8)

    xt = nc.declare_dram_parameter("xt", [D, S], BF16, isOutput=False)
    wq = nc.declare_dram_parameter("wq", [D, NCOL], BF16, isOutput=False)
    wk = nc.declare_dram_parameter("wk", [D, NCOL], BF16, isOutput=False)
    wv = nc.declare_dram_parameter("wv", [D, NCOL], BF16, isOutput=False)
    wo = nc.declare_dram_parameter("wo", [NCOL, D], BF16, isOutput=False)
    bqe = nc.declare_dram_parameter("bqe", [NCOL], F32, isOutput=False)
    maskb = nc.declare_dram_parameter("maskb", [NC2, skv], F32, isOutput=False)
    out = nc.declare_dram_parameter("out", [S, D], F16, isOutput=True)

    def ap3(param, ap_list, offset=0):
        p = param.ap()
        return AP(tensor=p.tensor, offset=p.offset + offset, ap=ap_list)

    with tile.TileContext(nc) as tc:
        with tc.tile_pool(name="singles", bufs=1) as singles:
            # --- persistent SBUF tiles -----------------------------------
            twqa = singles.tile([128, ND, NCOL], BF16, tag="wqa", name="twqa")
            twka = singles.tile([128, ND, NCOL], BF16, tag="wka", name="twka")
            twva = singles.tile([128, ND, NCOL], BF16, tag="wva", name="twva")
            twoa = singles.tile([128, NC2, D], BF16, tag="woa", name="twoa")
            # queries are host-permuted so the masked-in keys are columns
            # 0:skv of xt -- K/V projections read a prefix of the same tile
            txta = singles.tile([128, ND, S], BF16, tag="xta", name="txta")
            twq = [twqa[:, d, :] for d in range(ND)]
            twk = [twka[:, d, :] for d in range(ND)]
            twv = [twva[:, d, :] for d in range(ND)]
            two = [twoa[:, c, :] for c in range(NC2)]
            txkv = [txta[:, d, 0:skv] for d in range(ND)]
            txt = [
                [txta[:, d, i * 512:(i + 1) * 512] for i in range(NI)]
                for d in range(ND)
            ]
            tqt = [singles.tile([128, S], BF16, tag=f"qt{c}", name=f"qt{c}") for c in range(NC2)]
            tkt = [singles.tile([128, skv], BF16, tag=f"kt{c}", name=f"kt{c}") for c in range(NC2)]
            # V tiles: [j chunk, head, 80] with col 64 = ones (denominator)
            tva = singles.tile([128, nj, HPC, 80], BF16, tag="tva", name="tva")
            tot = [singles.tile([128, S], F32, tag=f"ot{c}", name=f"ot{c}") for c in range(NC2)]
            totn = [singles.tile([128, S], BF16, tag=f"otn{c}", name=f"otn{c}") for c in range(NC2)]
            # softmax denominators: rows 0/32/64/96 hold heads 0..3; the
            # custom-DVE reciprocal only works on full-width offset-0 APs,
            # so stage the rows and batch it
            tstage = singles.tile([97, S], F32, tag="lstage", name="tstage")
            trecf = singles.tile([97, S], F32, tag="lrecf", name="trecf")
            trecb = singles.tile([97, S], BF16, tag="lrecb", name="trecb")
            tones4 = singles.tile([97, 64], BF16, tag="ones4", name="tones4")
            tonesf = singles.tile([128, 64], F32, tag="onesf", name="tonesf")
            tmbs = singles.tile([128, NC2, nj], F32, tag="mbs", name="tmbs")
            tbq = singles.tile([128, NC2], F32, tag="tbq", name="tbq")

            # --- input DMAs ----------------------------------------------
            # Two ~200GB/s hardware DGE rings (sync, scalar).  Critical
            # chain is K proj (wk + all xtkv d-chunks) then Q proj of block
            # 0 (wq + xt cols 0:512); xtkv streams per-d on both rings so
            # the d-outer K-proj consumes chunks as they land.
            def dma_w(eng, tile_, param):
                eng.dma_start(
                    out=tile_,
                    in_=ap3(param, [[NCOL, 128], [128 * NCOL, ND], [1, NCOL]]),
                )

            def dma_xt(eng, dlo, dhi, clo, chi):
                if chi <= clo:
                    return
                eng.dma_start(
                    out=txta[:, dlo:dhi, clo:chi],
                    in_=ap3(
                        xt,
                        [[S, 128], [128 * S, dhi - dlo], [1, chi - clo]],
                        offset=dlo * 128 * S + clo,
                    ),
                )

            # column-chunk schedule: the first exp needs only wk + xt cols
            # 0:512 + half of wq + wv (~2.25MB over two ~110GB/s rings);
            # later key chunks and query blocks stream in behind it.
            colchunks = [(jo, jw) for jo, jw in kchunks]
            if skv < S:
                colchunks.append((skv, S - skv))
            # wk split by d-half so the first K matmul starts sooner
            nc.sync.dma_start(
                out=twka[:, 4:8, :],
                in_=ap3(wk, [[NCOL, 128], [128 * NCOL, 4], [1, NCOL]],
                        offset=4 * 128 * NCOL),
            )
            nc.sync.dma_start(
                out=twka[:, 0:4, :],
                in_=ap3(wk, [[NCOL, 128], [128 * NCOL, 4], [1, NCOL]]),
            )
            first = True
            for jo, jw in colchunks:
                dma_xt(nc.sync, 0, 4, jo, jo + jw)
                dma_xt(nc.scalar, 4, 8, jo, jo + jw)
                if first:
                    for c in range(NC2):
                        nc.sync.dma_start(
                            out=twqa[:, :, c * 128:(c + 1) * 128],
                            in_=ap3(
                                wq,
                                [[NCOL, 128], [128 * NCOL, ND], [1, 128]],
                                offset=c * 128,
                            ),
                        )
                        if c == 0:
                            dma_w(nc.scalar, twva, wv)
                    first = False
            nc.scalar.dma_start(
                out=twoa, in_=ap3(wo, [[D, 128], [128 * D, NC2], [1, D]])
            )
            # small tiles on the gpsimd (software DGE) queue
            nc.gpsimd.dma_start(
                out=tmbs, in_=ap3(maskb, [[1, 128], [skv, NC2], [128, nj]])
            )
            nc.gpsimd.dma_start(out=tbq, in_=ap3(bqe, [[1, 128], [128, NC2]]))
            tbiasq = [tbq[:, c:c + 1] for c in range(NC2)]

            # HAM pre-warm: the PE clock-gate defaults to 1.2GHz and takes
            # ~3.4us of sustained activity to release; burn dummy matmuls
            # on an uninitialized scratch tile while the input DMAs stream
            # so the K projection starts at 2.4GHz.  Results never read.
            tjunk = singles.tile([128, 512], BF16, tag="junk", name="tjunk")
            nc.vector.memset(tjunk, 1.0)
            nc.vector.memset(tonesf, 1.0)
            nc.vector.memset(tva, 1.0)  # cols 64:80 stay 1.0 -> ones column
            # rows of tstage between the 32h anchors are swept by the batched
            # reciprocal; init once so they stay finite
            nc.vector.memset(tstage, 1.0)
            for h in range(HPC):
                nc.vector.tensor_copy(
                    out=tones4[32 * h:32 * h + 1, :], in_=tonesf[0:1, :]
                )

            # --- phase 2: attention + normalize + output projection ------
            # The j-loop is paced by the ACT-engine exp; one pending unit of
            # independent PE work is popped per j iteration so the PE queue
            # never drains: V projection (early), Q projection of block i+1
            # and the output projection of block i-1.
            with (
                tc.tile_pool(name="pts", bufs=3) as pts,
                tc.tile_pool(name="obuf", bufs=4) as obuf,
                tc.tile_pool(name="ps2", bufs=2, space="PSUM") as ps2,
                tc.tile_pool(name="pot", bufs=2, space="PSUM") as pot,
                tc.tile_pool(name="plf", bufs=2, space="PSUM") as plf,
            ):
                pending = []
                state = {"tail": False}
                obs = {}
                dorder = [4, 5, 6, 7, 0, 1, 2, 3]  # DMA arrival order

                def emit_kc(ci, half):
                    # K projection of key chunk ci, d-half (0: d4-7 start,
                    # 1: d0-3 stop + bf16 cast).  Borrows plf slots.
                    jo, jw = kchunks[ci]
                    if half == 0:
                        state[f"kc{ci}"] = [
                            plf.tile([128, 512], F32, tag="plf", name="pkt")
                            for _ in range(NC2)
                        ]
                    kp = state[f"kc{ci}"]
                    for d in dorder[half * 4:half * 4 + 4]:
                        for c in range(NC2):
                            nc.tensor.matmul(
                                kp[c][:, 0:jw],
                                twk[d][:, c * 128:(c + 1) * 128],
                                txkv[d][:, jo:jo + jw],
                                start=(half == 0 and d == dorder[0]),
                                stop=(half == 1 and d == dorder[7]),
                            )
                    if half == 1:
                        for c in range(NC2):
                            nc.vector.tensor_copy(
                                out=tkt[c][:, jo:jo + jw], in_=kp[c][:, 0:jw]
                            )

                def emit_v(j):
                    # one V-proj unit: 8 accumulating matmuls -> fp8 cast
                    # into the per-head slices of tvd (borrows a plf slot)
                    ps = plf.tile([128, 512], F32, tag="plf", name="pvt")
                    for d in range(ND):
                        nc.tensor.matmul(
                            ps[:, 0:NCOL],
                            txkv[d][:, j * 128:(j + 1) * 128],
                            twv[d],
                            start=(d == 0),
                            stop=(d == ND - 1),
                        )
                    pin = AP(
                        tensor=ps.tensor, offset=ps.offset,
                        ap=[list(ps.ap[0]), [64, HPC], [1, 64]],
                    )
                    nc.vector.tensor_copy(out=tva[:, j, :, 0:64], in_=pin)

                def emit_pf(i, so, n):
                    # one outproj unit: 2 accumulating matmuls -> fp16 copy;
                    # DMA fires once both D-halves of the row block are done.
                    # In the drain phase ps2 is score-free, so odd units
                    # borrow its slots for a 4-deep PSUM pipeline.
                    sidx = i * 4 + so
                    ssl = slice(sidx * 128, (sidx + 1) * 128)
                    nsl = slice(n * 512, (n + 1) * 512)
                    if state["tail"] and so % 2:
                        pf = ps2.tile([128, 1024], F32, tag="ps", name="pft")[:, 0:512]
                    else:
                        pf = plf.tile([128, 512], F32, tag="plf", name="pft")
                    for c in range(NC2):
                        nc.tensor.matmul(
                            pf,
                            totn[c][:, ssl],
                            two[c][:, nsl],
                            start=(c == 0),
                            stop=(c == NC2 - 1),
                        )
                    if n == 0:
                        obs[so] = obuf.tile([128, 1024], F16, tag="ob", name="obt")
                    ob = obs[so]
                    if state["tail"] and (so + n) % 2:
                        # ACT is exp-free in the tail; split the copies so
                        # outproj units pipeline two-wide
                        nc.scalar.copy(out=ob[:, nsl], in_=pf)
                    else:
                        nc.vector.tensor_copy(out=ob[:, nsl], in_=pf)
                    if n == 1:
                        # odd blocks go to the gpsimd ring (scalar queue is
                        # busy with exp) except in the drain, where the
                        # scalar queue is free and gpsimd drains slowly
                        if sidx % 2 == 0:
                            ring = nc.sync
                        else:
                            ring = nc.scalar if state["tail"] else nc.gpsimd
                        ring.dma_start(out=out[ssl, :], in_=ob)

                def emit_norm(i):
                    # batched reciprocal + bf16 cast of 1/l, then per-head
                    # ones-row broadcast + normalize (deferred off the block
                    # boundary into later j-loop pop slots)
                    isl = slice(i * 512, (i + 1) * 512)
                    nc.vector.reciprocal_approx_fast(
                        out=trecf[:, isl], in_=tstage[:, isl]
                    )
                    nc.vector.tensor_copy(out=trecb[:, isl], in_=trecf[:, isl])
                    for c in range(NC2):
                        hA, hB = 2 * c, 2 * c + 1
                        plA = plf.tile([64, 512], F32, tag="plf", name="plA")
                        plB = plf.tile([64, 512], F32, tag="plf", name="plB")
                        nc.tensor.matmul(
                            plA,
                            tones4[32 * hA:32 * hA + 1, :],
                            trecb[32 * hA:32 * hA + 1, isl],
                            start=True,
                            stop=True,
                            tile_position=(32 * hA, 0),
                        )
                        nc.tensor.matmul(
                            plB,
                            tones4[32 * hB:32 * hB + 1, :],
                            trecb[32 * hB:32 * hB + 1, isl],
                            start=True,
                            stop=True,
                            tile_position=(32 * hB, 0),
                        )
                        nc.vector.tensor_mul(
                            out=totn[c][0:64, isl], in0=tot[c][0:64, isl], in1=plA
                        )
                        nc.vector.tensor_mul(
                            out=totn[c][64:128, isl], in0=tot[c][64:128, isl], in1=plB
                        )

                def emit_q(i, c):
                    # one Q-proj unit: 8 accumulating matmuls (borrowing a
                    # ps2 tile; scores j+1 uses the other buffer) + bias add
                    isl = slice(i * 512, (i + 1) * 512)
                    ps = ps2.tile([128, 1024], F32, tag="ps", name="pqt")
                    for d in range(ND):
                        nc.tensor.matmul(
                            ps[:, 0:512],
                            twq[d][:, c * 128:(c + 1) * 128],
                            txt[d][i],
                            start=(d == 0),
                            stop=(d == ND - 1),
                        )
                    nc.vector.tensor_scalar_add(
                        out=tqt[c][:, isl], in0=ps[:, 0:512], scalar1=tbiasq[c]
                    )

                # K projection of key chunk 0, V projection of chunks 0-3
                # and Q projection of (c=0, block 0) run before the first
                # j-loop; later K chunks, V chunks and the c=1 Q projection
                # interleave into (c=0, i=0)'s pop slots as they stream in.
                # Slot deadlines: kc(ci) before the j-loop reaches its first
                # key chunk; V(j) before iteration j (attnV j fires at j+1).
                pw = plf.tile([128, 512], F32, tag="plf", name="pwt")
                for _ in range(70):
                    nc.tensor.matmul(
                        pw[0:64, 0:64], tjunk[:, 0:64], tjunk[:, 0:64],
                        start=True, stop=True,
                    )
                emit_kc(0, 0)
                emit_kc(0, 1)
                for j in range(min(5, nj)):
                    emit_v(j)
                emit_q(0, 0)
                # warm slots with deadlines (slot k pops at end of c=0
                # iteration k, after attnV k-1): kc(ci) before its first
                # scores use; V(j) at slot <= j-1; Q0c1 last slot.
                warm = []
                for ci in range(1, len(kchunks)):
                    warm.append(lambda ci=ci: emit_kc(ci, 0))
                    warm.append(lambda ci=ci: emit_kc(ci, 1))
                wv_rest = [lambda j=j: emit_v(j) for j in range(5, nj)]
                if len(kchunks) == 3:
                    # interleave: [kc1a, kc1b, V5, V6, kc2a, kc2b, V7, ...]
                    head, tail2 = warm[:2], warm[2:]
                    warm = head + wv_rest[:2] + tail2 + wv_rest[2:]
                elif len(kchunks) > 3:
                    # degenerate skv (all-ones mask): run everything inline
                    for u in warm + wv_rest:
                        u()
                    warm = []
                else:
                    warm += wv_rest
                warm.append(lambda: emit_q(0, 1))

                for i in range(NI):
                    isl = slice(i * 512, (i + 1) * 512)
                    if i + 1 < NI:
                        # front of the queue: Q projections are the only
                        # dependency-free PE work at the block boundary,
                        # where scores stall ~800ns on the exp/ps2 chain
                        for c in reversed(range(NC2)):
                            pending.insert(
                                0, (i + 1, lambda i=i, c=c: emit_q(i + 1, c))
                            )
                    # force-emit units whose results this block depends on
                    # (Q projection of block i) if the queue hasn't drained
                    while any(dl <= i for dl, _ in pending):
                        pending.pop(0)[1]()
                    for c in range(NC2):
                        if i > 0 or c > 0:
                            while warm:
                                warm.pop(0)()
                        hA, hB = 2 * c, 2 * c + 1
                        potA = pot.tile([DH + 1, 512], F32, tag="pot", name="pott")
                        potB = pot.tile([DH + 1, 512], F32, tag="pot", name="pott")
                        pts_hist = []
                        for j in range(nj):
                            pscore = ps2.tile([128, 1024], F32, tag="ps", name="pscore")
                            nc.tensor.matmul(
                                pscore[:, 0:512],
                                tkt[c][0:64, j * 128:(j + 1) * 128],
                                tqt[c][0:64, isl],
                                start=True,
                                stop=True,
                                tile_position=(0, 0),
                            )
                            nc.tensor.matmul(
                                pscore[:, 512:1024],
                                tkt[c][64:128, j * 128:(j + 1) * 128],
                                tqt[c][64:128, isl],
                                start=True,
                                stop=True,
                                tile_position=(64, 0),
                            )
                            pt = pts.tile([128, 1024], BF16, tag="pt", name="ptile")
                            nc.scalar.activation(
                                out=pt,
                                in_=pscore,
                                func=AF.Exp,
                                bias=tmbs[:, c, j:j + 1],
                                scale=1.0 / math.sqrt(DH),
                            )
                            pts_hist.append(pt)
                            if j > 0:
                                pprev = pts_hist[j - 1]
                                nc.tensor.matmul(
                                    potA, tva[:, j - 1, hA, 0:65], pprev[:, 0:512],
                                    start=(j - 1 == 0), stop=False,
                                )
                                nc.tensor.matmul(
                                    potB, tva[:, j - 1, hB, 0:65], pprev[:, 512:1024],
                                    start=(j - 1 == 0), stop=False,
                                )
                            if warm:
                                warm.pop(0)()
                            elif pending:
                                pending.pop(0)[1]()
                        # one pop slot before the final pair: they wait on
                        # the last exp, so hand the PE independent work
                        if pending:
                            pending.pop(0)[1]()
                        nc.tensor.matmul(
                            potA, tva[:, nj - 1, hA, 0:65], pts_hist[nj - 1][:, 0:512],
                            start=(nj == 1), stop=True,
                        )
                        nc.tensor.matmul(
                            potB, tva[:, nj - 1, hB, 0:65], pts_hist[nj - 1][:, 512:1024],
                            start=(nj == 1), stop=True,
                        )
                        nc.vector.tensor_copy(out=tot[c][0:64, isl], in_=potA[0:DH, :])
                        nc.vector.tensor_copy(out=tot[c][64:128, isl], in_=potB[0:DH, :])
                        nc.vector.tensor_copy(
                            out=tstage[32 * hA:32 * hA + 1, isl],
                            in_=potA[DH:DH + 1, :],
                        )
                        nc.vector.tensor_copy(
                            out=tstage[32 * hB:32 * hB + 1, isl],
                            in_=potB[DH:DH + 1, :],
                        )
                    pending.append((NI + 1, lambda i=i: emit_norm(i)))
                    for so in range(4):
                        for n in range(2):
                            pending.append(
                                (NI + 1, lambda i=i, so=so, n=n: emit_pf(i, so, n))
                            )
                state["tail"] = True
                # the final norm chain leaves the PE idle ~4us, which
                # re-throttles HAM (K=4/8) and halves the drain's matmul
                # clock; after the norm unit is queued, keep the clock-gate
                # open with a few junk matmuls through the free pot slots
                for _ in range(6):
                    jp = plf.tile([128, 512], F32, tag="plf", name="jpt")
                    nc.tensor.matmul(
                        jp[0:64, :], tjunk[:, 0:64], tjunk, start=True, stop=True
                    )
                while pending:
                    pending.pop(0)[1]()

    nc.compile()
    return nc


def _get_program(skv):
    if skv not in _PROGRAM_CACHE:
        _PROGRAM_CACHE[skv] = build_program(skv)
    return _PROGRAM_CACHE[skv]


def _shard_inputs(hidden_states, attention_mask, Wq, bq, Wk, bk, Wv, bv,
                  emotion_w, Wo, bo):
    hs = np.asarray(hidden_states, dtype=np.float32)
    mask = np.asarray(attention_mask)
    Wq = np.asarray(Wq, dtype=np.float32)
    Wk = np.asarray(Wk, dtype=np.float32)
    Wv = np.asarray(Wv, dtype=np.float32)
    Wo = np.asarray(Wo, dtype=np.float32)
    bq = np.asarray(bq, dtype=np.float32)
    bv = np.asarray(bv, dtype=np.float32)
    bo = np.asarray(bo, dtype=np.float32)
    ew = np.asarray(emotion_w, dtype=np.float32)

    idx = [np.nonzero(mask[b])[0] for b in range(B)]
    sv = max(len(ix) for ix in idx)
    skv = max(128, ((sv + 127) // 128) * 128)



    in_maps = []
    perms = []
    for b in range(B):
        # permute queries so masked-in keys occupy columns 0:sv -- the K/V
        # projections then read a prefix of xt and no separate compacted
        # tensor is shipped.  Host inverse-permutes the output rows.
        rest = np.nonzero(mask[b] == 0)[0]
        perm = np.concatenate([idx[b], rest])
        perms.append(perm)
        xt_b = np.ascontiguousarray(hs[b][perm].T.astype(BF16_NP))  # [D, S]
        # -4 shift (softmax-invariant) keeps bf16 exp outputs small
        maskb_b = np.full((NC2, skv), -4.0, dtype=np.float32)
        maskb_b[:, len(idx[b]):] = -1e30
        for hg in range(H // HPC):
            cols = slice(hg * NCOL, (hg + 1) * NCOL)
            bqe = bq[cols] + ew[hg * HPC:(hg + 1) * HPC].reshape(NCOL)
            in_maps.append(
                {
                    "xt": xt_b,
                    "wq": np.ascontiguousarray(Wq[:, cols].astype(BF16_NP)),
                    "wk": np.ascontiguousarray(Wk[:, cols].astype(BF16_NP)),
                    "wv": np.ascontiguousarray(Wv[:, cols].astype(BF16_NP)),
                    "wo": np.ascontiguousarray(Wo[cols, :].astype(BF16_NP)),
                    "bqe": np.ascontiguousarray(bqe),
                    "maskb": maskb_b,
                }
            )
    # bk cancels in softmax; bv rides through the attention average:
    # out = attn @ (XWv) @ Wo + (bv @ Wo + bo)
    bo_adj = (bo.astype(np.float64) + bv.astype(np.float64) @ Wo.astype(np.float64))
    return in_maps, skv, bo_adj, perms


def run(inputs, trace=False, trace_kwargs=None):
    in_maps, skv, bo_adj, perms = _shard_inputs(**inputs)
    nc = _get_program(skv)
    res = run_bass_kernel_spmd(
        nc,
        in_maps,
        core_ids=list(range(8)),
        trace=trace,
        **(trace_kwargs or {}),
    )
    out = np.zeros((B, S, D), dtype=np.float32)
    for b in range(B):
        acc = np.zeros((S, D), dtype=np.float64)
        for hg in range(4):
            acc += res.results[b * 4 + hg]["out"].astype(np.float64)
        out[b][perms[b]] = (acc + bo_adj).astype(np.float32)
    return out, res


def kernel(**inputs):
    out, _ = run(inputs, trace=False)
    return out
